# revision 1
# baseline (speedup 1.0000x reference)
"""DN4 retrieval-kNN kernel for Trainium2 (8 NeuronCores, SPMD, no collectives).

Sharding: data-parallel over the 13230 flattened query-descriptor rows
(1654 rows -> 13 partition-tiles per core); the 5x2205-descriptor support
bank is replicated. Host adds the per-core partial (query, way) sums.

Per core:
  - support descriptors L2-normalized via diag-scaled PE transposes
    (rhs = diag(1/|y|) built by gpsimd.affine_select from a broadcast AP;
    norms: DVE squares+accum -> ACT sqrt -> DVE reciprocal)
  - query descriptors transposed raw; 1/|x| folded in AFTER top-k
    (a positive per-row scale preserves top-k selection)
  - sim = zqT.T @ desc on the PE in float32r (full-rate fp32, ~1e-4 rounding;
    produced by ACT copies which round PSUM fp32 -> SBUF float32r)
  - per-row top-8 via DVE max8 -- the pacing engine: 65 x (2205+58) cycles
    @0.96GHz ~= 153us; ACT PSUM->SBUF sim copies run just under it
  - main loop is way-outer / tile-inner so each way's descriptor bank is
    needed ~31us after the previous one; way-0's norm chain runs through
    psA's idle slots in the prologue (first max8 ~20us in), the remaining
    ways' transposes + GPSIMD diag builds trickle in the background
  - (query-slot, way) means via 13 tiny PE matmuls with a host-built
    row->slot mask (1/1323 entries) after the main loop
"""
import os
import sys

import numpy as np

for _p in ('/opt/trn_rl_repo', '/root/.axon_site/_ro/trn_rl_repo'):
    if os.path.isdir(_p) and _p not in sys.path:
        sys.path.insert(0, _p)

WAYS, SHOTS, Q = 5, 5, 30
C, HW = 128, 441
K = 3
NWAY = SHOTS * HW            # 2205 support descriptors per way
ND = WAYS * NWAY             # 11025
DT = 87                      # support descriptor row-tiles of 128
ND_PAD = DT * 128            # 11136
NCORES = 8
TROWS = Q * HW               # 13230 query-descriptor rows in total
RPC = (TROWS + NCORES - 1) // NCORES   # 1654 rows per core
MT = (RPC + 127) // 128      # 13 m-tiles per core
M_PAD = MT * 128             # 1664
SLOTS = 8                    # local query slots a core's rows can span (<=5)

CHUNKS_A = [(0, 512), (512, 512)]
CHUNKS_B = [(1024, 512), (1536, 512), (2048, 157)]

SG = 8                       # desc tiles per norm group
TG = 4                       # desc tiles per transpose PSUM tile (1 bank)
NG = (DT + SG - 1) // SG     # 11 norm groups

# desc norm groups needed before way w of the main loop can run:
# way w covers tiles floor(2205w/128) .. ceil(2205(w+1)/128)-1
def _groups_for_way(w):
    lo = (NWAY * w) // 128
    hi = (NWAY * (w + 1) + 127) // 128 - 1
    return lo // SG, hi // SG

_CACHE = {}


def _build_program():
    import concourse.bacc as bacc
    import concourse.mybir as mybir
    from concourse import tile

    dt = mybir.dt
    AF = mybir.ActivationFunctionType
    ALU = mybir.AluOpType
    AX = mybir.AxisListType

    nc = bacc.Bacc('TRN2', target_bir_lowering=False, debug=False)

    d_desc = nc.dram_tensor('desc', [128, DT * C], dt.float32, kind='ExternalInput')
    d_zq = nc.dram_tensor('zq', [128, MT * C], dt.float32, kind='ExternalInput')
    d_amask = nc.dram_tensor('amask', [128, MT * SLOTS], dt.float32, kind='ExternalInput')
    d_ident = nc.dram_tensor('ident', [128, 128], dt.float32, kind='ExternalInput')
    d_out = nc.dram_tensor('scores', [SLOTS, WAYS], dt.float32, kind='ExternalOutput')

    with tile.TileContext(nc) as tc:
        with tc.tile_pool(name='persist', bufs=1) as pp, \
             tc.tile_pool(name='work', bufs=3) as wp, \
             tc.tile_pool(name='sim', bufs=3) as simp:

            desc3 = pp.tile([128, DT, C], dt.float32)
            ddiag = pp.tile([128, DT, C], dt.float32)
            D_r = pp.tile([128, DT, C], dt.float32r)
            zq3 = pp.tile([128, MT, C], dt.float32)
            ZQ_r = pp.tile([128, MT, C], dt.float32r)
            amask3 = pp.tile([128, MT, SLOTS], dt.float32)
            ident = pp.tile([128, 128], dt.float32)
            ssd = pp.tile([128, DT], dt.float32)
            rnd = pp.tile([128, DT], dt.float32)
            ssq = pp.tile([128, MT], dt.float32)
            rnq = pp.tile([128, MT], dt.float32)
            junk_gp = pp.tile([128, C], dt.float32)
            junk_act = pp.tile([128, C], dt.float32)
            junk_dv = pp.tile([128, C], dt.float32)
            junk_dve = pp.tile([128, K], dt.float32)
            sqd = pp.tile([128, DT], dt.float32)
            sqq = pp.tile([128, MT], dt.float32)
            tiny = pp.tile([128, 1], dt.float32)
            stvals = pp.tile([128, WAYS, MT], dt.float32)
            scsb = pp.tile([SLOTS, MT, WAYS], dt.float32)
            scout = pp.tile([SLOTS, WAYS], dt.float32)

            # ---- input DMAs (critical-path slices first) ----
            half = (MT // 2) * C
            nc.sync.dma_start(desc3[:, 0:SG, :], d_desc[:, 0:SG * C])
            nc.sync.dma_start(zq3[:, 0:MT // 2, :], d_zq[:, 0:half])
            nc.sync.dma_start(ident[:], d_ident[:])
            for g in range(SG, 3 * SG, SG):
                nc.sync.dma_start(desc3[:, g:g + SG, :],
                                  d_desc[:, g * C:(g + SG) * C])
            nc.sync.dma_start(zq3[:, MT // 2:MT, :], d_zq[:, half:MT * C])
            for g in range(3 * SG, DT, SG):
                ge = min(g + SG, DT)
                nc.sync.dma_start(desc3[:, g:ge, :], d_desc[:, g * C:ge * C])
            nc.sync.dma_start(amask3[:], d_amask[:])

            nc.gpsimd.memset(tiny[:], 1e-24)

            with tc.tile_pool(name='pst', bufs=1, space='PSUM') as pst, \
                 tc.tile_pool(name='psA', bufs=2, space='PSUM') as psA, \
                 tc.tile_pool(name='psB', bufs=1, space='PSUM') as psB:

                def norm_group(gi, with_affine=True):
                    """Norms for desc tiles [gi*SG, gi*SG+SG): squares (DVE) ->
                    sqrt (ACT) -> recip (DVE) -> diag tiles (GPSIMD)."""
                    g = gi * SG
                    ge = min(g + SG, DT)
                    for t in range(g, ge):
                        # GPSIMD can't run ALU ops on HW; squares live on DVE
                        # in the DMA-bound prologue
                        nc.vector.scalar_tensor_tensor(
                            junk_dv[:], desc3[:, t, :], 1.0, desc3[:, t, :],
                            op0=ALU.mult, op1=ALU.mult,
                            accum_out=ssd[:, t:t + 1])
                    nc.scalar.activation(sqd[:, g:ge], ssd[:, g:ge], AF.Sqrt,
                                         bias=tiny[:])
                    nc.vector.reciprocal(rnd[:, g:ge], sqd[:, g:ge])
                    if with_affine:
                        affine(g, ge)

                def affine(g, ge):
                    # ddiag[p, t, f] = rnd[p, t] if p == f else 0 (gpsimd)
                    rn_b = rnd[:, g:ge].unsqueeze(2).broadcast_to([128, ge - g, C])
                    nc.gpsimd.affine_select(
                        ddiag[:, g:ge, :], rn_b, pattern=[[0, ge - g], [-1, C]],
                        compare_op=ALU.is_equal, fill=0.0,
                        base=0, channel_multiplier=1)

                def build_group(gi, early=False):
                    """Diag-scaled transposes + PSUM->SBUF copies for a group.
                    Early groups borrow psA's idle 2-bank slots (bufs=2) so the
                    way-0 prologue pipelines; background groups trickle through
                    the single-bank pst pool."""
                    g = gi * SG
                    ge = min(g + SG, DT)
                    for u in range(g, ge, TG):
                        ue = min(u + TG, ge)
                        if early:
                            pt = psA.tile([128, TG, C], dt.float32, tag='pa')
                        else:
                            pt = pst.tile([128, TG, C], dt.float32, tag='pt')
                        for j in range(ue - u):
                            t = u + j
                            nc.tensor.matmul(pt[:, j, :], desc3[:, t, :],
                                             ddiag[:, t, :], start=True, stop=True)
                        nc.scalar.activation(D_r[:, u:ue, :], pt[:, 0:ue - u, :],
                                             AF.Copy)

                def zq_unit(g):
                    ge = min(g + TG, MT)
                    pt = pst.tile([128, TG, C], dt.float32, tag='pt')
                    for j in range(ge - g):
                        nc.tensor.matmul(pt[:, j, :], zq3[:, g + j, :], ident[:],
                                         start=True, stop=True)
                    nc.scalar.activation(ZQ_r[:, g:ge, :], pt[:, 0:ge - g, :], AF.Copy)

                zq_unit(0)  # t=0..3 only; later units interleave into way 0

                # ---- way-0 desc chain first (the latency-critical path) ----
                glo0, ghi0 = _groups_for_way(0)
                done = set()
                for gi in range(glo0, ghi0 + 1):
                    norm_group(gi, with_affine=False)
                affine(glo0 * SG, min((ghi0 + 1) * SG, DT))  # one batched op
                for g in range(TG, MT, TG):
                    zq_unit(g)  # remaining query transposes, off the hot loop
                for gi in range(glo0, ghi0 + 1):
                    build_group(gi, early=True)
                    done.add(gi)
                # query norms (only needed by the epilogue rnq fold)
                for t in range(MT):
                    nc.vector.scalar_tensor_tensor(
                        junk_dv[:], zq3[:, t, :], 1.0, zq3[:, t, :],
                        op0=ALU.mult, op1=ALU.mult, accum_out=ssq[:, t:t + 1])
                nc.scalar.activation(sqq[:], ssq[:], AF.Sqrt, bias=tiny[:])
                nc.vector.reciprocal(rnq[:], sqq[:])
                # remaining descriptor norms (DMA-paced background)
                for gi in range(NG):
                    if gi not in (0, 1, 2):
                        norm_group(gi)

                # ---- main loop: way-outer / tile-inner ----
                Dflat = D_r[:].rearrange("p t c -> p (t c)")
                for w in range(WAYS):
                    base = w * NWAY
                    m8big = wp.tile([128, MT, 8], dt.float32, tag='m8')
                    for t in range(MT):
                        lhsT = ZQ_r[:, t, :]
                        pa = psA.tile([128, 1024], dt.float32, tag='pa')
                        pb = psB.tile([128, 1181], dt.float32, tag='pb')
                        for off, sz in CHUNKS_A:
                            nc.tensor.matmul(pa[:, off:off + sz], lhsT,
                                             Dflat[:, base + off:base + off + sz],
                                             start=True, stop=True)
                        for off, sz in CHUNKS_B:
                            if sz % 2:  # ragged tail: odd N fails fp32r ISA check
                                nc.tensor.matmul(
                                    pb[:, off - 1024:off - 1024 + sz],
                                    lhsT.bitcast(dt.float32),
                                    Dflat[:, base + off:base + off + sz].bitcast(dt.float32),
                                    start=True, stop=True)
                            else:
                                nc.tensor.matmul(pb[:, off - 1024:off - 1024 + sz],
                                                 lhsT,
                                                 Dflat[:, base + off:base + off + sz],
                                                 start=True, stop=True)
                        sim = simp.tile([128, NWAY], dt.float32, tag='sim')
                        nc.scalar.activation(sim[:, 0:1024], pa[:], AF.Copy)
                        nc.scalar.activation(sim[:, 1024:NWAY], pb[:], AF.Copy)
                        nc.vector.max(m8big[:, t, :], sim[:])
                    # per-way top-3 sums for all tiles in one reduce
                    nc.vector.reduce_sum(stvals[:, w, :], m8big[:, :, 0:K],
                                         axis=AX.X)
                    # emit the NEXT way's transposes right after this way's
                    # matmuls -- they execute during way w+1's 33us window
                    if w + 1 < WAYS:
                        glo, ghi = _groups_for_way(w + 1)
                        for gi in range(glo, ghi + 1):
                            if gi not in done:
                                build_group(gi)
                                done.add(gi)

            # ---- fold m-rows into (query, way) scores ----
            rq_b = rnq[:].unsqueeze(1).broadcast_to([128, WAYS, MT])
            nc.vector.tensor_tensor(stvals[:], stvals[:], rq_b, op=ALU.mult)
            with tc.tile_pool(name='psS', bufs=1, space='PSUM') as psS:
                scps = psS.tile([SLOTS, MT, WAYS], dt.float32)
                for t in range(MT):
                    nc.tensor.matmul(scps[0:SLOTS, t, :], amask3[:, t, :],
                                     stvals[:, :, t], start=True, stop=True)
                nc.scalar.activation(scsb[:], scps[:], AF.Copy)
            nc.vector.reduce_sum(scout[:], scsb[:].transpose([0, 2, 1]), axis=AX.X)
            nc.sync.dma_start(d_out[:], scout[:])

    nc.finalize()
    return nc


def _host_prep(support_images, support_labels, query_images):
    support_images = np.ascontiguousarray(np.asarray(support_images, np.float32))
    support_labels = np.asarray(support_labels, np.float32)
    query_images = np.ascontiguousarray(np.asarray(query_images, np.float32))

    labels = np.argmax(support_labels, axis=1)
    order = np.argsort(labels, kind='stable')
    sup = support_images[order].reshape(WAYS * SHOTS, C, HW)

    desc_byrow = sup.transpose(0, 2, 1).reshape(ND, C)
    desc_byrow = np.concatenate(
        [desc_byrow, np.zeros((ND_PAD - ND, C), np.float32)], 0)
    desc_dev = desc_byrow.reshape(DT, 128, C).transpose(1, 0, 2).reshape(128, DT * C)
    desc_dev = np.ascontiguousarray(desc_dev)

    # flat query-descriptor rows [13230, C], row r = (q = r//441, hw = r%441)
    zq_flat = query_images.reshape(Q, C, HW).transpose(0, 2, 1).reshape(TROWS, C)
    zq_devs, amask_devs = [], []
    for core in range(NCORES):
        r0 = core * RPC
        zb = zq_flat[r0:r0 + RPC]
        zb = np.concatenate(
            [zb, np.zeros((M_PAD - zb.shape[0], C), np.float32)], 0)
        zq_devs.append(np.ascontiguousarray(
            zb.reshape(MT, 128, C).transpose(1, 0, 2).reshape(128, MT * C)))
        q0 = r0 // HW
        amask = np.zeros((128, MT, SLOTS), np.float32)
        lr = np.arange(MT * 128)
        r = r0 + lr
        valid = (lr < RPC) & (r < TROWS)
        amask[lr[valid] % 128, lr[valid] // 128, (r[valid] // HW) - q0] = \
            1.0 / (HW * K)
        amask_devs.append(np.ascontiguousarray(amask.reshape(128, MT * SLOTS)))
    ident = np.ascontiguousarray(np.eye(128, dtype=np.float32))
    return desc_dev, zq_devs, amask_devs, ident


def kernel(support_images, support_labels, query_images):
    from concourse import bass_utils

    if 'nc' not in _CACHE:
        _CACHE['nc'] = _build_program()
    nc = _CACHE['nc']

    desc_dev, zq_devs, amask_devs, ident = _host_prep(
        support_images, support_labels, query_images)

    in_maps = [{'desc': desc_dev, 'zq': zq_devs[c],
                'amask': amask_devs[c], 'ident': ident} for c in range(NCORES)]
    try:
        res = bass_utils.run_bass_kernel_spmd(
            nc, in_maps, core_ids=list(range(NCORES)))
    except Exception:
        # transient NRT/tunnel failures happen; one retry
        import time
        time.sleep(2.0)
        res = bass_utils.run_bass_kernel_spmd(
            nc, in_maps, core_ids=list(range(NCORES)))
    scores = np.zeros((Q, WAYS), np.float32)
    for c in range(NCORES):
        q0 = (c * RPC) // HW
        part = res.results[c]['scores']
        for s in range(SLOTS):
            if q0 + s < Q:
                scores[q0 + s] += part[s]
    return scores.astype(np.float32)



# revision 22
# speedup vs baseline: 1.3888x; 1.3888x over previous
"""DN4 retrieval-kNN kernel for Trainium2 (8 NeuronCores, SPMD, no collectives).

Sharding: data-parallel over the 13230 flattened query-descriptor rows
(1654 rows -> 13 partition-tiles per core); the 5x2205-descriptor support
bank is replicated. Host adds the per-core partial (query, way) sums.

v2 design (cost-model-driven):
  - descriptors are L2-normalized AND transposed on the host; fp16 device
    inputs halve DMA and feed the PE directly (no on-device norm chain or
    transposes at all)
  - sim = zqT.T @ descT on the PE in fp16 (1 cyc/col), PSUM fp32
  - per-(way, m-tile) top-3: PSUM is drained by a mix of
      * type A units: ACT converts PSUM fp32 -> SBUF fp16 (2 copies),
        then DVE runs a pairwise tensor_tensor-max fold cascade at the
        2x 16-bit rate down to 138 candidates + one max8
      * type B units: DVE folds PSUM fp32 pairs directly (no ACT),
        then the same fp16 cascade
    the A:B ratio balances ACT (~2.2us/unit) against DVE (~1.5/2.3us)
  - pairwise max folds are top-3-lossy only when two of a row's top-3
    collide in the same fold chain (~2% of (row,way) pairs, error
    ~gap/3 ~ 1e-4 absolute on a ~0.3 score; tolerance is 2e-2)
  - (query-slot, way) means via 13 accumulating PE matmuls with a
    host-built row->slot mask (amask) after the main loop
"""
import os
import sys

import numpy as np

for _p in ('/opt/trn_rl_repo', '/root/.axon_site/_ro/trn_rl_repo'):
    if os.path.isdir(_p) and _p not in sys.path:
        sys.path.insert(0, _p)

WAYS, SHOTS, Q = 5, 5, 30
C, HW = 128, 441
K = 3
NWAY = SHOTS * HW            # 2205 support descriptors per way
ND = WAYS * NWAY             # 11025
NCORES = 8
TROWS = Q * HW               # 13230 query-descriptor rows in total
RPC = (TROWS + NCORES - 1) // NCORES   # 1654 rows per core
MT = (RPC + 127) // 128      # 13 m-tiles per core
M_PAD = MT * 128             # 1664
SLOTS = 8                    # local query slots a core's rows can span (<=5)

# PSUM split: psA [128,1024] (2 banks, single-buffered), psB [128,1181]
# (3 banks, double-buffered) = 8 banks. psB double-buffering lets ACT
# run ahead through the B-unit DVE spikes.
NA = 1024
NB = NWAY - NA               # 1181
PER_WAY_SCORES = False
# fold-cascade widths: 2205 ->1103 ->552 ->276 ->138 -> max8
F1, F2, F3, F4 = 1103, 552, 276, 138

# units (w, t) whose PSUM is drained by DVE tensor_tensor folds (type B);
# the rest are converted to fp16 by ACT (type A). Cascades are emitted
# PEND_DEPTH units late so the next units' PSUM drains (which gate the
# single psA buffer and ACT) jump ahead in DVE's in-order queue.
B_TILES = (0, 4, 8)
# explicit unit-index set (overrides B_TILES): 13 evenly spread units
B_UNITS = frozenset((int(i * 65 / 13) + 4) % 65 for i in range(13))
PEND_DEPTH = 1
SBUF_BUFS = 4
WARMUP_MM = 4

_CACHE = {}


def _build_program(b_tiles=B_TILES):
    import concourse.bacc as bacc
    import concourse.mybir as mybir
    from concourse import tile

    dt = mybir.dt
    AF = mybir.ActivationFunctionType
    ALU = mybir.AluOpType
    AX = mybir.AxisListType

    nc = bacc.Bacc('TRN2', target_bir_lowering=False, debug=False)

    d_desc = nc.dram_tensor('desc', [128, ND], dt.float16, kind='ExternalInput')
    d_zq = nc.dram_tensor('zq', [128, MT * C], dt.float16, kind='ExternalInput')
    d_amask = nc.dram_tensor('amask', [128, MT * SLOTS], dt.float32,
                             kind='ExternalInput')
    d_out = nc.dram_tensor('scores', [SLOTS, WAYS], dt.float32,
                           kind='ExternalOutput')

    with tile.TileContext(nc) as tc:
        with tc.tile_pool(name='persist', bufs=1) as pp, \
             tc.tile_pool(name='sim', bufs=SBUF_BUFS) as simp, \
             tc.tile_pool(name='fold1', bufs=SBUF_BUFS) as fp1, \
             tc.tile_pool(name='fold2', bufs=SBUF_BUFS) as fp2, \
             tc.tile_pool(name='fold3', bufs=SBUF_BUFS) as fp3, \
             tc.tile_pool(name='fold4', bufs=SBUF_BUFS) as fp4p, \
             tc.tile_pool(name='m8', bufs=2) as m8p:

            descT = pp.tile([128, WAYS, NWAY], dt.float16)
            zqT = pp.tile([128, MT, C], dt.float16)
            amask = pp.tile([128, MT, SLOTS], dt.float32)
            stvals = pp.tile([128, WAYS, MT], dt.float32)
            scout = pp.tile([SLOTS, WAYS], dt.float32)

            # critical-path first: tile-0 queries, way-0 bank halves, then
            # the remaining queries / ways / amask
            nc.sync.dma_start(zqT[:, 0, :], d_zq[:, 0:C])
            nc.sync.dma_start(descT[:, 0, 0:NA], d_desc[:, 0:NA])
            nc.sync.dma_start(descT[:, 0, NA:NWAY], d_desc[:, NA:NWAY])
            nc.sync.dma_start(zqT[:, 1:MT, :], d_zq[:, C:MT * C])
            for w in range(1, WAYS):
                nc.sync.dma_start(descT[:, w, :],
                                  d_desc[:, w * NWAY:(w + 1) * NWAY])
            nc.sync.dma_start(amask[:], d_amask[:])

            wsrc = pp.tile([128, 512], dt.float16)
            nc.gpsimd.memset(wsrc[:], 0.0)

            with tc.tile_pool(name='psA', bufs=1, space='PSUM') as psA, \
                 tc.tile_pool(name='psB', bufs=2, space='PSUM') as psB:
                # PE p-state warm-up: keep the tensor engine continuously
                # busy from t=0 so the first real matmuls run at full clock
                # (the cost model ramps PE speed with continuous-busy time)
                if WARMUP_MM:
                    wps = psA.tile([128, NA], dt.float32, tag='pa')
                    for _ in range(WARMUP_MM):
                        nc.tensor.matmul(wps[:, 0:512], wsrc[:, 0:128],
                                         wsrc[:], start=True, stop=True)
                pend = []
                m8bigs = {}
                for ui in range(WAYS * MT):
                    w, t = divmod(ui, MT)
                    if t == 0:
                        m8bigs[w] = m8p.tile([128, MT, 8], dt.float16,
                                             tag='m8', name='m8big')
                    m8big = m8bigs[w]
                    lhsT = zqT[:, t, :]
                    Dw = descT[:, w, :]
                    pa = psA.tile([128, NA], dt.float32, tag='pa')
                    pb = psB.tile([128, NB], dt.float32, tag='pb')
                    is_b = (ui in B_UNITS) if B_UNITS is not None \
                        else t in b_tiles

                    def mm_pa(pa=pa, lhsT=lhsT, Dw=Dw):
                        nc.tensor.matmul(pa[:, 0:512], lhsT, Dw[:, 0:512],
                                         start=True, stop=True)
                        nc.tensor.matmul(pa[:, 512:NA], lhsT,
                                         Dw[:, 512:NA],
                                         start=True, stop=True)

                    def mm_pb(pb=pb, lhsT=lhsT, Dw=Dw):
                        nc.tensor.matmul(pb[:, 0:512], lhsT,
                                         Dw[:, NA:NA + 512],
                                         start=True, stop=True)
                        nc.tensor.matmul(pb[:, 512:1024], lhsT,
                                         Dw[:, NA + 512:NA + 1024],
                                         start=True, stop=True)
                        nc.tensor.matmul(pb[:, 1024:NB], lhsT,
                                         Dw[:, NA + 1024:NWAY],
                                         start=True, stop=True)

                    if is_b or ui == 0:
                        mm_pa(), mm_pb()
                    else:
                        mm_pb(), mm_pa()

                    if is_b:
                        # type E: ACT converts pa while DVE takes top-8 of
                        # pb straight from PSUM (HW allows one PSUM input
                        # per DVE op, so pairwise PSUM folds are illegal).
                        # ACT work in this window (op_pa here + op_pb of
                        # the next unit) matches DVE's (max8 + cascade).
                        sim16 = simp.tile([128, NWAY], dt.float16,
                                          tag='sim16')
                        g1 = fp2.tile([128, F2], dt.float16, tag='f2')
                        nc.vector.max(g1[:, 512:520], pb[:])
                        nc.scalar.activation(sim16[:, 0:NA], pa[:], AF.Copy)

                        def cascade(sim16=sim16, g1=g1, m8big=m8big,
                                    t=t, w=w):
                            nc.vector.tensor_tensor(
                                g1[:, 0:512], sim16[:, 0:512],
                                sim16[:, 512:NA], op=ALU.max)
                            g2 = fp3.tile([128, F3], dt.float16, tag='f3')
                            nc.vector.tensor_tensor(
                                g2[:, 0:260], g1[:, 0:260], g1[:, 260:520],
                                op=ALU.max)
                            g3 = fp4p.tile([128, F4], dt.float16, tag='f4')
                            nc.vector.tensor_tensor(
                                g3[:, 0:130], g2[:, 0:130], g2[:, 130:260],
                                op=ALU.max)
                            nc.vector.max(m8big[:, t, :], g3[:, 0:130])
                            if t == MT - 1:
                                nc.vector.reduce_sum(stvals[:, w, :],
                                                     m8big[:, :, 0:K],
                                                     axis=AX.X)

                        pend.append(cascade)
                        if len(pend) > PEND_DEPTH:
                            pend.pop(0)()
                        continue
                    f1 = fp1.tile([128, F1], dt.float16, tag='f1')
                    if True:
                        # type A: ACT converts fp32 -> fp16 (pb first: it
                        # is ready early thanks to psB double-buffering)
                        sim16 = simp.tile([128, NWAY], dt.float16,
                                          tag='sim16')
                        nc.scalar.activation(sim16[:, NA:NWAY], pb[:],
                                             AF.Copy)
                        nc.scalar.activation(sim16[:, 0:NA], pa[:], AF.Copy)

                        def casc0(f1=f1, sim16=sim16):
                            nc.vector.tensor_tensor(
                                f1[:], sim16[:, 0:F1],
                                sim16[:, NWAY - F1:NWAY], op=ALU.max)

                    def cascade(f1=f1, m8big=m8big, t=t, w=w, casc0=casc0):
                        if casc0 is not None:
                            casc0()
                        f2 = fp2.tile([128, F2], dt.float16, tag='f2')
                        nc.vector.tensor_tensor(
                            f2[:], f1[:, 0:F2], f1[:, F1 - F2:F1], op=ALU.max)
                        f3 = fp3.tile([128, F3], dt.float16, tag='f3')
                        nc.vector.tensor_tensor(
                            f3[:], f2[:, 0:F3], f2[:, F2 - F3:F2], op=ALU.max)
                        f4 = fp4p.tile([128, F4], dt.float16, tag='f4')
                        nc.vector.tensor_tensor(
                            f4[:], f3[:, 0:F4], f3[:, F3 - F4:F3], op=ALU.max)
                        nc.vector.max(m8big[:, t, :], f4[:])
                        if t == MT - 1:
                            # way complete: top-3 sums in one reduce
                            nc.vector.reduce_sum(stvals[:, w, :],
                                                 m8big[:, :, 0:K], axis=AX.X)

                    pend.append(cascade)
                    if len(pend) > PEND_DEPTH:
                        pend.pop(0)()
                for c in pend:
                    c()

                # ---- fold m-rows into (query, way) scores; reuse the pa
                # bank region (avoids a pool-transition drain barrier) ----
                scps = psA.tile([128, NA], dt.float32, tag='pa')
                for t in range(MT):
                    nc.tensor.matmul(scps[0:SLOTS, 0:WAYS], amask[:, t, :],
                                     stvals[:, :, t],
                                     start=(t == 0), stop=(t == MT - 1))
                nc.scalar.activation(scout[:], scps[0:SLOTS, 0:WAYS], AF.Copy)
            nc.sync.dma_start(d_out[:], scout[:])

    nc.finalize()
    return nc


def _host_prep(support_images, support_labels, query_images):
    support_images = np.asarray(support_images, np.float32)
    support_labels = np.asarray(support_labels, np.float32)
    query_images = np.asarray(query_images, np.float32)

    labels = np.argmax(support_labels, axis=1)
    order = np.argsort(labels, kind='stable')
    sup = support_images[order].reshape(WAYS * SHOTS, C, HW)

    desc = sup.transpose(0, 2, 1).reshape(ND, C)
    desc = desc / np.maximum(
        np.linalg.norm(desc, axis=1, keepdims=True), 1e-12)
    desc_dev = np.ascontiguousarray(desc.T.astype(np.float16))  # [128, ND]

    zq = query_images.reshape(Q, C, HW).transpose(0, 2, 1).reshape(TROWS, C)
    zq = zq / np.maximum(np.linalg.norm(zq, axis=1, keepdims=True), 1e-12)

    zq_devs, amask_devs = [], []
    for core in range(NCORES):
        r0 = core * RPC
        zb = zq[r0:r0 + RPC]
        zb = np.concatenate(
            [zb, np.zeros((M_PAD - zb.shape[0], C), np.float32)], 0)
        # device layout [128 C-partitions, MT tiles x 128 rows]
        zt = zb.reshape(MT, 128, C).transpose(2, 0, 1).reshape(128, MT * 128)
        zq_devs.append(np.ascontiguousarray(zt.astype(np.float16)))
        q0 = r0 // HW
        amask = np.zeros((128, MT, SLOTS), np.float32)
        lr = np.arange(MT * 128)
        r = r0 + lr
        valid = (lr < RPC) & (r < TROWS)
        amask[lr[valid] % 128, lr[valid] // 128, (r[valid] // HW) - q0] = \
            1.0 / (HW * K)
        amask_devs.append(np.ascontiguousarray(amask.reshape(128, MT * SLOTS)))
    return desc_dev, zq_devs, amask_devs


def kernel(support_images, support_labels, query_images):
    from concourse import bass_utils

    if 'nc' not in _CACHE:
        _CACHE['nc'] = _build_program()
    nc = _CACHE['nc']

    desc_dev, zq_devs, amask_devs = _host_prep(
        support_images, support_labels, query_images)

    in_maps = [{'desc': desc_dev, 'zq': zq_devs[c], 'amask': amask_devs[c]}
               for c in range(NCORES)]
    try:
        res = bass_utils.run_bass_kernel_spmd(
            nc, in_maps, core_ids=list(range(NCORES)))
    except Exception:
        # transient NRT/tunnel failures happen; one retry
        import time
        time.sleep(2.0)
        res = bass_utils.run_bass_kernel_spmd(
            nc, in_maps, core_ids=list(range(NCORES)))
    scores = np.zeros((Q, WAYS), np.float32)
    for c in range(NCORES):
        q0 = (c * RPC) // HW
        part = res.results[c]['scores']
        for s in range(SLOTS):
            if q0 + s < Q:
                scores[q0 + s] += part[s]
    return scores.astype(np.float32)
